# revision 15
# baseline (speedup 1.0000x reference)
"""BiMamba (bidirectional Mamba-1 block) on 8 Trainium2 NeuronCores.

Sharding: 8 cores = (batch 2) x (direction 2) x (d_inner half 2). Every core
runs an identical SPMD program on host-prepared data (backward-direction cores
receive the time-flipped sequence, so the device program is direction-free).
The only cross-core communication is a per-chunk AllReduce of the x_proj
partial sums between the two d_inner-half cores of each (batch, direction).

Device layout is feature-major: [channels/partitions, time/free]. The
selective scan runs as hardware linear-recurrence instructions
(tensor_tensor_scan) over [128, 1024] tiles, one per (channel-block, state).
The n-state contraction with C is done by accumulating identity matmuls in
PSUM. The depthwise causal conv runs as 4 diagonal-stationary matmuls with
shifted inputs.

kernel(**inputs) takes the FULL unsharded inputs and returns (out, masks)
exactly like the reference.
"""

import os

os.environ.setdefault("MYCRO_LOCAL_CACHE", "1")

import numpy as np
import ml_dtypes

import concourse.bass as bass
import concourse.bacc as bacc
import concourse.tile as tile
import concourse.mybir as mybir

F32 = mybir.dt.float32
BF16 = mybir.dt.bfloat16
F16 = mybir.dt.float16
AF = mybir.ActivationFunctionType
OP = mybir.AluOpType
BF = ml_dtypes.bfloat16

# Problem geometry (nn_BiMambaModule: D_MODEL=1024, EXPAND=2, D_STATE=16,
# D_CONV=4, DT_RANK=64, B=2, L=4096), hardcoded per the harness contract.
B = 2
L = 4096
DM = 1024          # d_model
DI = 2048          # d_inner
CH = 1024          # d_inner half handled per core
NST = 16           # d_state
KC = 4             # conv kernel
DTR = 64           # dt_rank
NPROJ = DTR + 2 * NST  # 96

P = 128            # partitions
NB = CH // P       # 8 channel blocks per core
NK = DM // P       # 8 contraction chunks for in_proj
NCHUNK = 4
LC = L // NCHUNK   # 1024 time steps per chunk
HF = LC // 2       # 512 matmul free-dim half

REPLICA_GROUPS = [[0, 1], [2, 3], [4, 5], [6, 7]]

# CoreSim has no Silu table; x*sigmoid(x) decomposition is used when False.
USE_FUSED_SILU = True


def _emit(tc, io):
    from contextlib import ExitStack
    nc = tc.nc
    es = ExitStack()

    # ---------------- persistent pools ----------------
    wp = es.enter_context(tc.tile_pool(name="weights", bufs=1))
    pp = es.enter_context(tc.tile_pool(name="persist", bufs=1))

    wxi = []
    wz = []
    wo = []
    xpw = []
    for k in range(NK):
        t = wp.tile([P, CH], BF16, name=f"wxi{k}", tag=f"wxi{k}")
        nc.sync.dma_start(t[:], io["wxiT"][k * P:(k + 1) * P, :])
        wxi.append(t)
        t = wp.tile([P, CH], BF16, name=f"wz{k}", tag=f"wz{k}")
        nc.sync.dma_start(t[:], io["wzT"][k * P:(k + 1) * P, :])
        wz.append(t)
        t = wp.tile([P, DM], BF16, name=f"wo{k}", tag=f"wo{k}")
        nc.sync.dma_start(t[:], io["woT"][k * P:(k + 1) * P, :])
        wo.append(t)
        t = wp.tile([P, NPROJ], BF16, name=f"xpw{k}", tag=f"xpw{k}")
        nc.sync.dma_start(t[:], io["xpwT"][k * P:(k + 1) * P, :])
        xpw.append(t)
    dtw = wp.tile([DTR, CH], BF16, name="dtw", tag="dtw")
    nc.sync.dma_start(dtw[:], io["dtwT"][:, :])
    convd = []
    for j in range(KC):
        row = []
        for m in range(NB):
            t = wp.tile([P, P], BF16, name=f"convd{j}_{m}", tag=f"convd{j}_{m}")
            nc.sync.dma_start(t[:], io["convd"][j, m, :, :])
            row.append(t)
        convd.append(row)
    ident = wp.tile([P, P], BF16, name="ident", tag="ident")
    nc.sync.dma_start(ident[:], io["ident"][:, :])

    convb = pp.tile([P, NB], F32, name="convb", tag="convb")
    nc.sync.dma_start(convb[:], io["convb"][:, :])
    negdtb = pp.tile([P, NB], F32, name="negdtb", tag="negdtb")
    nc.sync.dma_start(negdtb[:], io["negdtb"][:, :])
    dvec = pp.tile([P, NB], F32, name="dvec", tag="dvec")
    nc.sync.dma_start(dvec[:], io["dvec"][:, :])
    negA = pp.tile([P, NB * NST], F32, name="negA", tag="negA")
    nc.sync.dma_start(negA[:], io["negA"][:, :])

    # carries: per block a [P, NST] tile of scan states, one column per n
    carry = []
    for m in range(NB):
        t = pp.tile([P, NST], F32, name=f"carry{m}", tag=f"carry{m}")
        carry.append(t)
    # conv halos: last 3 columns of xi from the previous chunk
    halo = []
    for m in range(NB):
        t = pp.tile([P, KC - 1], BF16, name=f"halo{m}", tag=f"halo{m}")
        nc.vector.memset(t[:], 0.0)
        halo.append(t)

    # ---------------- working pools ----------------
    s1 = es.enter_context(tc.tile_pool(name="stage1", bufs=1))
    sx = es.enter_context(tc.tile_pool(name="xtiles", bufs=1))
    sp = es.enter_context(tc.tile_pool(name="scan", bufs=2))
    sh = es.enter_context(tc.tile_pool(name="scanh", bufs=2))
    sm = es.enter_context(tc.tile_pool(name="scanm", bufs=2))
    sbc = es.enter_context(tc.tile_pool(name="bcast", bufs=2))
    so = es.enter_context(tc.tile_pool(name="outp", bufs=1))
    ssil = es.enter_context(tc.tile_pool(name="silu", bufs=1))
    sq = es.enter_context(tc.tile_pool(name="small", bufs=1))
    ps = es.enter_context(tc.tile_pool(name="psum", bufs=2, space="PSUM"))
    psy = es.enter_context(tc.tile_pool(name="psumy", bufs=2, space="PSUM"))
    dr = es.enter_context(tc.tile_pool(name="dram", bufs=2, space="DRAM"))

    for c in range(NCHUNK):
        t0 = c * LC

        # ---- load xT chunk ----
        xT = []
        for k in range(NK):
            t = sx.tile([P, LC], BF16, name=f"xT{k}_{c}", tag=f"xT{k}")
            nc.sync.dma_start(t[:], io["xT"][k * P:(k + 1) * P, t0:t0 + LC])
            xT.append(t)

        # ---- in_proj: xi (with conv halo layout) and z -> silu(z) ----
        xi = []
        silz = []
        for m in range(NB):
            acc = ps.tile([P, LC], F32, name=f"ipx{m}_{c}", tag="psA")
            for f in range(2):
                for k in range(NK):
                    nc.tensor.matmul(
                        acc[:, f * HF:(f + 1) * HF],
                        wxi[k][:, m * P:(m + 1) * P],
                        xT[k][:, f * HF:(f + 1) * HF],
                        start=(k == 0), stop=(k == NK - 1))
            xit = sx.tile([P, KC - 1 + LC], BF16, name=f"xi{m}_{c}", tag=f"xi{m}")
            nc.vector.tensor_copy(xit[:, 0:KC - 1], halo[m][:])
            nc.vector.tensor_copy(xit[:, KC - 1:KC - 1 + LC], acc[:])
            # stash tail for next chunk's halo
            nc.vector.tensor_copy(halo[m][:], xit[:, LC:LC + KC - 1])
            xi.append(xit)
        for m in range(NB):
            acc = ps.tile([P, LC], F32, name=f"ipz{m}_{c}", tag="psA")
            for f in range(2):
                for k in range(NK):
                    nc.tensor.matmul(
                        acc[:, f * HF:(f + 1) * HF],
                        wz[k][:, m * P:(m + 1) * P],
                        xT[k][:, f * HF:(f + 1) * HF],
                        start=(k == 0), stop=(k == NK - 1))
            szt = s1.tile([P, LC], BF16, name=f"silz{m}_{c}", tag=f"silz{m}")
            if USE_FUSED_SILU:
                nc.scalar.activation(szt[:], acc[:], AF.Silu)
            else:
                zt = ssil.tile([P, LC], BF16, name=f"zt{m}_{c}", tag="zt")
                nc.scalar.activation(zt[:], acc[:], AF.Identity)
                sgz = ssil.tile([P, LC], BF16, name=f"sgz{m}_{c}", tag="sgz")
                nc.scalar.activation(sgz[:], acc[:], AF.Sigmoid)
                nc.vector.tensor_tensor(szt[:], zt[:], sgz[:], op=OP.mult)
            silz.append(szt)

        # ---- depthwise causal conv + silu -> xt ----
        xt = []
        for m in range(NB):
            acc = ps.tile([P, LC], F32, name=f"cv{m}_{c}", tag="psA")
            for f in range(2):
                for j in range(KC):
                    nc.tensor.matmul(
                        acc[:, f * HF:(f + 1) * HF],
                        convd[j][m][:],
                        xi[m][:, j + f * HF:j + f * HF + HF],
                        start=(j == 0), stop=(j == KC - 1))
            xtt = s1.tile([P, LC], BF16, name=f"xt{m}_{c}", tag=f"xt{m}")
            if USE_FUSED_SILU:
                nc.scalar.activation(xtt[:], acc[:], AF.Silu,
                                     bias=convb[:, m:m + 1])
            else:
                xb = ssil.tile([P, LC], BF16, name=f"xb{m}_{c}", tag="zt")
                nc.scalar.activation(xb[:], acc[:], AF.Identity,
                                     bias=convb[:, m:m + 1])
                sgx = ssil.tile([P, LC], BF16, name=f"sgx{m}_{c}", tag="sgz")
                nc.scalar.activation(sgx[:], acc[:], AF.Sigmoid,
                                     bias=convb[:, m:m + 1])
                nc.vector.tensor_tensor(xtt[:], xb[:], sgx[:], op=OP.mult)
            xt.append(xtt)

        # ---- x_proj partial + AllReduce over the d_inner-half pair ----
        accp = ps.tile([NPROJ, LC], F32, name=f"xp_{c}", tag="psA")
        for f in range(2):
            for k in range(NB):
                nc.tensor.matmul(
                    accp[:, f * HF:(f + 1) * HF],
                    xpw[k][:],
                    xt[k][:, f * HF:(f + 1) * HF],
                    start=(k == 0), stop=(k == NB - 1))
        cc_in = dr.tile([NPROJ, LC], F32, name=f"ccin_{c}", tag="ccin")
        cc_out = dr.tile([NPROJ, LC], F32, name=f"ccout_{c}", tag="ccout")
        proj = sq.tile([NPROJ, LC], F32, name=f"pr_{c}", tag="proj")
        nc.vector.tensor_copy(proj[:], accp[:])
        nc.sync.dma_start(cc_in[:], proj[:])
        nc.gpsimd.collective_compute(
            "AllReduce", OP.add, replica_groups=REPLICA_GROUPS,
            ins=[cc_in.opt()], outs=[cc_out.opt()])
        nc.sync.dma_start(proj[:], cc_out[:])

        # ---- B/C rows to bf16, bounce via DRAM for broadcast reads ----
        bcrows = sq.tile([2 * NST, LC], BF16, name=f"bcr_{c}", tag="bcrows")
        nc.vector.tensor_copy(bcrows[:], proj[DTR:NPROJ, :])
        bc_dram = dr.tile([2 * NST, LC], BF16, name=f"bcd_{c}", tag="bcd")
        nc.sync.dma_start(bc_dram[:], bcrows[:])

        # ---- dt path: l = -softplus(dt_logits + dt_bias) (F16) ----
        dtl = sq.tile([DTR, LC], BF16, name=f"dtl_{c}", tag="dtl")
        nc.vector.tensor_copy(dtl[:], proj[0:DTR, :])
        lmu = []
        wmu = []
        for m in range(NB):
            acc = ps.tile([P, LC], F32, name=f"dt{m}_{c}", tag="psA")
            for f in range(2):
                nc.tensor.matmul(
                    acc[:, f * HF:(f + 1) * HF],
                    dtw[:, m * P:(m + 1) * P],
                    dtl[:, f * HF:(f + 1) * HF],
                    start=True, stop=True)
            lt = s1.tile([P, LC], F16, name=f"l{m}_{c}", tag=f"l{m}")
            nc.scalar.activation(lt[:], acc[:], AF.Sigmoid, scale=-1.0,
                                 bias=negdtb[:, m:m + 1])
            nc.scalar.activation(lt[:], lt[:], AF.Ln)
            lmu.append(lt)
            # w = dt * u = (-l) * xt
            wt = s1.tile([P, LC], BF16, name=f"w{m}_{c}", tag=f"w{m}")
            nc.vector.scalar_tensor_tensor(
                wt[:], lt[:], -1.0, xt[m][:], op0=OP.mult, op1=OP.mult)
            wmu.append(wt)

        # ---- selective scan: four sweeps of 2 channel-blocks ----
        ys = []
        for sweep in range(4):
            blks = range(sweep * 2, sweep * 2 + 2)
            ypss = {}
            for m in blks:
                ypss[m] = psy.tile([P, LC], F32, name=f"yps{m}_{c}", tag="psY")
            for n in range(NST):
                bcB = sbc.tile([P, LC], BF16, name=f"bcB{n}_{c}_{sweep}",
                               tag="bcB")
                rap = bc_dram[n:n + 1, :]
                nc.sync.dma_start(
                    bcB[:], bass.AP(rap.tensor, rap.offset,
                                    [[0, P]] + rap.ap[1:]))
                bcC = sbc.tile([P, LC], BF16, name=f"bcC{n}_{c}_{sweep}",
                               tag="bcC")
                rap = bc_dram[NST + n:NST + n + 1, :]
                nc.sync.dma_start(
                    bcC[:], bass.AP(rap.tensor, rap.offset,
                                    [[0, P]] + rap.ap[1:]))
                for m in blks:
                    dA = sp.tile([P, LC], F32, name=f"dA{m}_{n}_{c}", tag="dA")
                    nc.scalar.activation(dA[:], lmu[m][:], AF.Exp,
                                         scale=negA[:, m * NST + n:m * NST + n + 1])
                    q = sp.tile([P, LC], BF16, name=f"q{m}_{n}_{c}", tag="q")
                    nc.vector.tensor_tensor(q[:], wmu[m][:], bcB[:],
                                            op=OP.mult)
                    h = sh.tile([P, LC], BF16, name=f"h{m}_{n}_{c}", tag="h")
                    init = 0.0 if c == 0 else carry[m][:, n:n + 1]
                    nc.vector.tensor_tensor_scan(
                        h[:], dA[:], q[:], init, op0=OP.mult, op1=OP.add)
                    nc.vector.tensor_copy(carry[m][:, n:n + 1],
                                          h[:, LC - 1:LC])
                    mt = sm.tile([P, LC], BF16, name=f"m{m}_{n}_{c}", tag="mt")
                    nc.vector.tensor_tensor(mt[:], h[:], bcC[:], op=OP.mult)
                    for f in range(2):
                        nc.tensor.matmul(
                            ypss[m][:, f * HF:(f + 1) * HF],
                            ident[:],
                            mt[:, f * HF:(f + 1) * HF],
                            start=(n == 0), stop=(n == NST - 1))
            # y_final = (y + xt*D) * silu(z), written in place of silz
            for m in blks:
                yd = sm.tile([P, LC], BF16, name=f"yd{m}_{c}", tag="yd")
                nc.vector.scalar_tensor_tensor(
                    yd[:], xt[m][:], dvec[:, m:m + 1], ypss[m][:],
                    op0=OP.mult, op1=OP.add)
                nc.vector.tensor_tensor(silz[m][:], yd[:], silz[m][:],
                                        op=OP.mult)
                ys.append(silz[m])

        # ---- out_proj (transposed output, host re-transposes) ----
        for mo in range(NB):
            acc = ps.tile([P, LC], F32, name=f"op{mo}_{c}", tag="psA")
            for f in range(2):
                for k in range(NB):
                    nc.tensor.matmul(
                        acc[:, f * HF:(f + 1) * HF],
                        wo[k][:, mo * P:(mo + 1) * P],
                        ys[k][:, f * HF:(f + 1) * HF],
                        start=(k == 0), stop=(k == NB - 1))
            oT = so.tile([P, LC], F32, name=f"oT{mo}_{c}", tag="oT")
            nc.scalar.copy(oT[:], acc[:])
            nc.sync.dma_start(io["outT"][mo * P:(mo + 1) * P, t0:t0 + LC],
                              oT[:])

    es.close()


def build_program():
    nc = bacc.Bacc("TRN2", target_bir_lowering=False, debug=False,
                   enable_asserts=False, num_devices=8)
    io = {}
    io["xT"] = nc.dram_tensor("xT", [DM, L], BF16, kind="ExternalInput")
    io["wxiT"] = nc.dram_tensor("wxiT", [DM, CH], BF16, kind="ExternalInput")
    io["wzT"] = nc.dram_tensor("wzT", [DM, CH], BF16, kind="ExternalInput")
    io["woT"] = nc.dram_tensor("woT", [CH, DM], BF16, kind="ExternalInput")
    io["xpwT"] = nc.dram_tensor("xpwT", [CH, NPROJ], BF16,
                                kind="ExternalInput")
    io["dtwT"] = nc.dram_tensor("dtwT", [DTR, CH], BF16, kind="ExternalInput")
    io["convd"] = nc.dram_tensor("convd", [KC, NB, P, P], BF16,
                                 kind="ExternalInput")
    io["ident"] = nc.dram_tensor("ident", [P, P], BF16, kind="ExternalInput")
    io["convb"] = nc.dram_tensor("convb", [P, NB], F32, kind="ExternalInput")
    io["negdtb"] = nc.dram_tensor("negdtb", [P, NB], F32,
                                  kind="ExternalInput")
    io["dvec"] = nc.dram_tensor("dvec", [P, NB], F32, kind="ExternalInput")
    io["negA"] = nc.dram_tensor("negA", [P, NB * NST], F32,
                                kind="ExternalInput")
    io["outT"] = nc.dram_tensor("outT", [CH, L], F32, kind="ExternalOutput")

    with tile.TileContext(nc) as tc:
        _emit(tc, io)
    nc.compile()
    return nc


def prep_core_inputs(inputs, b, d, h):
    """Build the in_map for core (batch b, direction d, half h)."""
    f32 = np.float32
    x = np.asarray(inputs["x"], f32)[b]          # (L, DM)
    if d == 1:
        x = x[::-1]
    xT = np.ascontiguousarray(x.T).astype(BF)     # (DM, L)

    ipw = np.asarray(inputs["in_proj_w"], f32)    # (2*DI, DM)
    wxiT = np.ascontiguousarray(ipw[h * CH:(h + 1) * CH, :].T).astype(BF)
    wzT = np.ascontiguousarray(ipw[DI + h * CH:DI + (h + 1) * CH, :].T).astype(BF)

    sfx = "" if d == 0 else "_b"
    cw = np.asarray(inputs["conv_w" + sfx], f32)[h * CH:(h + 1) * CH, 0, :]
    cb = np.asarray(inputs["conv_b" + sfx], f32)[h * CH:(h + 1) * CH]
    xpw = np.asarray(inputs["x_proj_w" + sfx], f32)[:, h * CH:(h + 1) * CH]
    dtwn = "dt_proj_w" + sfx
    dtbn = "dt_proj_b" + sfx
    aln = "A_log" if d == 0 else "A_b_log"
    dn = "D" if d == 0 else "D_b"
    dtw = np.asarray(inputs[dtwn], f32)[h * CH:(h + 1) * CH, :]   # (CH, DTR)
    dtb = np.asarray(inputs[dtbn], f32)[h * CH:(h + 1) * CH]
    alog = np.asarray(inputs[aln], f32)[h * CH:(h + 1) * CH, :]   # (CH, NST)
    dv = np.asarray(inputs[dn], f32)[h * CH:(h + 1) * CH]
    opw = np.asarray(inputs["out_proj_w"], f32)[:, h * CH:(h + 1) * CH]

    convd = np.zeros((KC, NB, P, P), f32)
    for j in range(KC):
        for m in range(NB):
            np.fill_diagonal(convd[j, m], cw[m * P:(m + 1) * P, j])

    def pm(v):  # (CH,) -> (P, NB) column-per-block
        return np.ascontiguousarray(v.reshape(NB, P).T)

    negA = np.exp(alog)                           # = -A, (CH, NST)
    negA_pm = np.zeros((P, NB * NST), f32)
    for m in range(NB):
        negA_pm[:, m * NST:(m + 1) * NST] = negA[m * P:(m + 1) * P, :]

    return {
        "xT": xT,
        "wxiT": wxiT,
        "wzT": wzT,
        "woT": np.ascontiguousarray(opw.T).astype(BF),     # (CH, DM)
        "xpwT": np.ascontiguousarray(xpw.T).astype(BF),    # (CH, NPROJ)
        "dtwT": np.ascontiguousarray(dtw.T).astype(BF),    # (DTR, CH)
        "convd": convd.astype(BF),
        "ident": np.eye(P, dtype=BF),
        "convb": pm(cb).astype(f32),
        "negdtb": pm(-dtb).astype(f32),
        "dvec": pm(dv).astype(f32),
        "negA": negA_pm.astype(f32),
    }


def assemble_output(inputs, partials):
    """partials: list of 8 outT arrays (CH, L) fp32, core order
    (b, d, h) = b*4 + d*2 + h. Returns (out, masks)."""
    f32 = np.float32
    out = np.zeros((B, L, DM), f32)
    for b in range(B):
        acc = np.zeros((L, DM), f32)
        for d in range(2):
            s = np.zeros((L, DM), f32)
            for h in range(2):
                s += np.asarray(partials[b * 4 + d * 2 + h], f32).T
            if d == 1:
                s = s[::-1]
            acc += s
        out[b] = acc
    masks = np.asarray(inputs["masks"])
    out = np.where(masks[:, 0, :, None], out, 0.0).astype(f32)
    return out, masks


_CACHE = {}


def run(inputs, trace=False, **spmd_kwargs):
    """Compile (cached), execute on the 8 NeuronCores, and assemble.
    Returns ((out, masks), BassKernelResults)."""
    if "nc" not in _CACHE:
        _CACHE["nc"] = build_program()
    nc = _CACHE["nc"]

    in_maps = []
    for b in range(B):
        for d in range(2):
            for h in range(2):
                in_maps.append(prep_core_inputs(inputs, b, d, h))

    from concourse.bass_utils import run_bass_kernel_spmd
    res = run_bass_kernel_spmd(nc, in_maps, list(range(8)), trace=trace,
                               **spmd_kwargs)
    partials = [res.results[i]["outT"] for i in range(8)]
    return assemble_output(inputs, partials), res


def kernel(**inputs):
    (out, masks), _ = run(inputs)
    return out, masks
